# revision 6
# baseline (speedup 1.0000x reference)
"""Distributed gaussian-mask attention for trn2 (8 NeuronCores, SPMD).

Problem: B=2, S=2048, H=1024, 16 heads, hd=64.
  q/k/v = x@W*, dif = q - k, score = exp(-0.5 * dif @ dif^T),
  prob = score * triu(ones,k=1), ctx = prob @ v, out = ctx @ Wo + bo.
  (bq/bk/bv are zeros by construction -- folded out; dif = x @ (Wq-Wk).)

Sharding (uniform SPMD program, data-only per-core differences):
  - Head parallel: core c owns heads (2c, 2c+1) = 128 feature columns of
    Wq/Wk/Wv.  Each core computes D^T = (Wq-Wk)c^T-proj and V for ALL
    tokens of its 2 heads, runs the full (anti-)causal attention
    triangle locally (no collective), producing ctx^T [128, 4096].
  - FOUR AllToAlls (one per batch-half, 256 KB each) re-shard ctx from
    head-major to token-major as soon as each half batch of ctx^T is
    ready, overlapping the collectives with attention compute.  Core c
    ends with full-H ctx^T for tokens {b, h*1024 + [128c, 128c+128)};
    it then runs its 1/8 of the output projection with the full Wo.

Perf structure (v2):
  - QB=512 query blocks (4 per batch) -> N=512 matmuls, fewer per-inst
    overheads.
  - dT stored fp16: score matmuls run at 1 col/cycle (vs 2 for f32r)
    and get FWL on the weight load.
  - The two heads' score MMs run CONCURRENTLY on the PE via row-group
    tiling (K=64 each at row offsets 0/64); the two ctx MMs run
    concurrently via col-group tiling (M=64 at col offsets 0/64).
  - exp is split across engines: head 0 on ACT (spline Exp), head 1 on
    DVE via the Schraudolph bitcast trick:
      exp(y) ~ bitcast_f32(int32(y * 2^23/ln2 + (127*2^23 - 366000)))
    (max ~3% sawtooth; end-to-end rel-err simulated at 3.7e-3).
  - Software pipelining: score pair of iteration i+1 is emitted before
    the ctx pair of iteration i so the PE never waits on the exp.
  - batch-0's output projection is drip-fed (4 MMs per attention
    iteration) into the batch-1 attention stream; only batch-1's
    out-projection (plus its half-2 AllToAll) remains on the tail.

Precision: x/Wd/Wv fp16, dT fp16, score PSUM fp32, prob bf16, V bf16,
  ctx PSUM fp32, ctx bf16, Wo bf16, out fp32.
"""
import numpy as np
import ml_dtypes

import concourse.bass as bass
import concourse.bacc as bacc
import concourse.mybir as mybir
import concourse.tile as tile
from concourse.bass_utils import run_bass_kernel_spmd

FP = mybir.dt.float32
F16 = mybir.dt.float16
BF = mybir.dt.bfloat16
I32 = mybir.dt.int32
AF = mybir.ActivationFunctionType
ALU = mybir.AluOpType

NC = 8
B, S, H, NH, HD = 2, 2048, 1024, 16, 64
T = B * S            # 4096 tokens
QB = 512             # query block
KB = 128             # key block
NQB = S // QB        # 4 query blocks per batch
NKB = S // KB        # 16 key blocks per batch
OSLOT = 128          # out-projection tokens per core per (batch, half)

# Schraudolph fast-exp constants: exp(-0.5*x) ~ bitcast(int(A*x + Bc))
EXP_A = float(np.float32(-0.5 * (1 << 23) / np.log(2)))
EXP_B = float(np.float32(127 * (1 << 23) - 366000.0))

_cached = {}


def _build(dbg=False):
    nc = bacc.Bacc("TRN2", target_bir_lowering=False, debug=False, num_devices=NC)

    xT = nc.dram_tensor("xT", [H, T], F16, kind="ExternalInput")
    Wqc = nc.dram_tensor("Wqc", [H, 128], FP, kind="ExternalInput")
    Wkc = nc.dram_tensor("Wkc", [H, 128], FP, kind="ExternalInput")
    Wvc = nc.dram_tensor("Wvc", [H, 128], F16, kind="ExternalInput")
    Wob = nc.dram_tensor("Wob", [H, H], BF, kind="ExternalInput")
    bo_d = nc.dram_tensor("bo", [H], FP, kind="ExternalInput")
    mask_d = nc.dram_tensor("maskbf", [128, 128], BF, kind="ExternalInput")
    out_d = nc.dram_tensor("out", [H, 4 * OSLOT], FP, kind="ExternalOutput")
    if dbg:
        dbg_dT = nc.dram_tensor("dbg_dT", [8 * 128, 512], F16, kind="ExternalOutput")
        dbg_V = nc.dram_tensor("dbg_V", [8 * 128, 512], BF, kind="ExternalOutput")
        dbg_ctxT = nc.dram_tensor(
            "dbg_ctxT", [B * 128, 2048], BF, kind="ExternalOutput"
        )

    with tile.TileContext(nc) as tc:
        with (
            tc.tile_pool(name="res", bufs=1) as res,      # resident SBUF
            tc.tile_pool(name="stream", bufs=3) as strm,  # streamed SBUF
            tc.tile_pool(name="dram", bufs=1, space="DRAM") as dram,
        ):
            # ---------------- constants / weights in ----------------
            mask_t = res.tile([128, 128], BF, tag="mask")
            nc.sync.dma_start(mask_t[:], mask_d[:])
            bo_t = res.tile([128, 8], FP, tag="bo")
            nc.sync.dma_start(bo_t[:], bo_d[:].rearrange("(f p) -> p f", p=128))

            wd = []
            wv = []
            wo = []
            for k in range(8):
                wq_k = strm.tile([128, 128], FP, tag="wqk", name=f"wq{k}")
                wk_k = strm.tile([128, 128], FP, tag="wkk", name=f"wk{k}")
                nc.sync.dma_start(wq_k[:], Wqc[k * 128:(k + 1) * 128, :])
                nc.sync.dma_start(wk_k[:], Wkc[k * 128:(k + 1) * 128, :])
                wd_k = res.tile([128, 128], F16, tag=f"wd{k}", name=f"wd{k}")
                nc.vector.tensor_sub(wd_k[:], wq_k[:], wk_k[:])
                wd.append(wd_k)
                wv_k = res.tile([128, 128], F16, tag=f"wv{k}", name=f"wv{k}")
                nc.sync.dma_start(wv_k[:], Wvc[k * 128:(k + 1) * 128, :])
                wv.append(wv_k)
                wo_k = res.tile([128, 1024], BF, tag=f"wo{k}", name=f"wo{k}")
                nc.sync.dma_start(wo_k[:], Wob[k * 128:(k + 1) * 128, :])
                wo.append(wo_k)

            # resident outputs of the projections
            dT = [res.tile([128, 512], F16, tag=f"dT{i}", name=f"dT{i}")
                  for i in range(8)]                  # D^T  [128 feat, 4096 tok]
            Vg = [res.tile([128, 512], BF, tag=f"Vg{i}", name=f"Vg{i}")
                  for i in range(8)]                  # V    [tok, feat] 4 tiles/grp

            # ---------------- projections: D^T and V ----------------
            with tc.tile_pool(name="psp", bufs=1, space="PSUM") as psp:
                for half in range(2):                 # token halves (2048 each)
                    xk_tiles = []
                    for k in range(8):
                        xk = strm.tile([128, 2048], F16, tag="xk", name=f"xk{half}_{k}")
                        nc.sync.dma_start(
                            xk[:], xT[k * 128:(k + 1) * 128,
                                      half * 2048:(half + 1) * 2048]
                        )
                        xk_tiles.append(xk)
                    pd = [psp.tile([128, 512], FP, tag=f"pd{j}", name=f"pd{half}_{j}")
                          for j in range(4)]
                    pv = [psp.tile([128, 512], FP, tag=f"pv{j}", name=f"pv{half}_{j}")
                          for j in range(4)]
                    for k in range(8):
                        xk = xk_tiles[k]
                        for j in range(4):            # 512-token chunks -> D^T
                            nc.tensor.matmul(
                                pd[j][:], wd[k][:], xk[:, j * 512:(j + 1) * 512],
                                start=(k == 0), stop=(k == 7),
                            )
                        for t in range(16):           # 128-token tiles -> V
                            nc.tensor.matmul(
                                pv[t // 4][:, (t % 4) * 128:(t % 4 + 1) * 128],
                                xk[:, t * 128:(t + 1) * 128], wv[k][:],
                                start=(k == 0 and t % 4 == 0), stop=(k == 7),
                                skip_group_check=True,
                            )
                    for j in range(4):
                        nc.vector.tensor_copy(dT[half * 4 + j][:], pd[j][:])
                        nc.vector.tensor_copy(Vg[half * 4 + j][:], pv[j][:])
                if dbg:
                    for i in range(8):
                        nc.sync.dma_start(
                            dbg_dT[i * 128:(i + 1) * 128, :], dT[i][:]
                        )
                        nc.sync.dma_start(
                            dbg_V[i * 128:(i + 1) * 128, :], Vg[i][:]
                        )

            # ---------------- attention (local, 2 heads) ----------------
            ctxT = [res.tile([128, 2048], BF, tag=f"ctxT{b}", name=f"ctxT{b}")
                    for b in range(B)]
            # received full-H ctx tiles: [b][half] -> 8 x [128, 128]
            ctxg = [[None, None], [None, None]]

            with (
                tc.tile_pool(name="pss", bufs=2, space="PSUM") as pss,
                tc.tile_pool(name="pcx", bufs=2, space="PSUM") as pcx,
                tc.tile_pool(name="pso", bufs=2, space="PSUM") as pso,
            ):
                # ---- AllToAll for one (batch, token-half): 256 KB ----
                def fire_a2a(b, h):
                    cc_in = dram.tile([1024, OSLOT], BF, name=f"cc_in{b}_{h}")
                    cc_out = dram.tile([1024, OSLOT], BF, name=f"cc_out{b}_{h}")
                    for j in range(8):
                        nc.sync.dma_start(
                            cc_in[j * 128:(j + 1) * 128, :],
                            ctxT[b][:, h * 1024 + j * OSLOT:
                                      h * 1024 + (j + 1) * OSLOT],
                        )
                    nc.gpsimd.collective_compute(
                        "AllToAll",
                        mybir.AluOpType.bypass,
                        replica_groups=[list(range(NC))],
                        ins=[cc_in[:].opt()],
                        outs=[cc_out[:].opt()],
                    )
                    gs = []
                    for k in range(8):
                        g = res.tile([128, OSLOT], BF, tag=f"cg{b}_{h}_{k}",
                                     name=f"cg{b}_{h}_{k}")
                        nc.sync.dma_start(g[:], cc_out[k * 128:(k + 1) * 128, :])
                        gs.append(g)
                    ctxg[b][h] = gs

                # ---- out-projection, drip-fed one MM at a time ----
                oq = []          # pending (b, h, fo, k) micro-ops
                ostate = {}

                def emit_outproj_mm():
                    if not oq:
                        return
                    b, h, fo, k = oq.pop(0)
                    if k == 0:
                        ostate["po"] = pso.tile([128, OSLOT], FP, tag="po",
                                                name=f"po{b}_{h}_{fo}")
                    po = ostate["po"]
                    nc.tensor.matmul(
                        po[:], wo[k][:, fo * 128:(fo + 1) * 128],
                        ctxg[b][h][k][:],
                        start=(k == 0), stop=(k == 7),
                        skip_group_check=True,
                    )
                    if k == 7:
                        ot = strm.tile([128, OSLOT], FP, tag="ot", bufs=4,
                                       name=f"ot{b}_{h}_{fo}")
                        nc.vector.tensor_scalar_add(
                            ot[:], po[:], bo_t[:, fo:fo + 1]
                        )
                        nc.sync.dma_start(
                            out_d[fo * 128:(fo + 1) * 128,
                                  (2 * b + h) * OSLOT:(2 * b + h + 1) * OSLOT],
                            ot[:],
                        )

                # ---- score pair (both heads, concurrent row tiles) ----
                def emit_score(b, qb, kb):
                    qt = b * 4 + qb                   # dT tile of this q block
                    koff = b * S + kb * KB
                    kt, kc = koff // 512, koff % 512
                    j = kb - 4 * qb
                    n = 128 * (j + 1) if j < 4 else QB
                    ps0 = pss.tile([128, QB], FP, tag="ps0",
                                   name=f"ps0_{b}_{qb}_{kb}")
                    ps1 = pss.tile([128, QB], FP, tag="ps1",
                                   name=f"ps1_{b}_{qb}_{kb}")
                    nc.tensor.matmul(
                        ps0[:, 0:n], dT[kt][0:64, kc:kc + 128],
                        dT[qt][0:64, 0:n], start=True, stop=True,
                    )
                    nc.tensor.matmul(
                        ps1[:, 0:n], dT[kt][64:128, kc:kc + 128],
                        dT[qt][64:128, 0:n], start=True, stop=True,
                    )
                    return ps0, ps1, j, n

                # ---- exp pair: head0 on ACT, head1 on DVE fast-exp ----
                def emit_exp(b, qb, kb, ps0, ps1, j, n):
                    at0 = strm.tile([128, QB], BF, tag="at0", bufs=3,
                                    name=f"at0_{b}_{qb}_{kb}")
                    at1 = strm.tile([128, QB], BF, tag="at1", bufs=3,
                                    name=f"at1_{b}_{qb}_{kb}")
                    it1 = strm.tile([128, QB], I32, tag="it1", bufs=2,
                                    name=f"it1_{b}_{qb}_{kb}")
                    nc.scalar.activation(at0[:, 0:n], ps0[:, 0:n], AF.Exp,
                                         scale=-0.5)
                    nc.vector.tensor_scalar(
                        it1[:, 0:n], ps1[:, 0:n], EXP_A, EXP_B,
                        ALU.mult, ALU.add,
                    )
                    def fs(a, bnd):
                        return it1[:, a:bnd].bitcast(FP)
                    if j < 4:                         # diagonal: mask last 128
                        nc.vector.tensor_mul(
                            at0[:, j * 128:n], at0[:, j * 128:n], mask_t[:]
                        )
                        if j > 0:
                            nc.vector.tensor_copy(
                                at1[:, 0:j * 128], fs(0, j * 128)
                            )
                        nc.vector.tensor_mul(
                            at1[:, j * 128:n], fs(j * 128, n), mask_t[:]
                        )
                    else:
                        nc.vector.tensor_copy(at1[:, 0:n], fs(0, n))
                    return at0, at1

                # ---- ctx pair (both heads, concurrent col tiles) ----
                def emit_ctx(b, qb, kb, pc, at0, at1, n):
                    g, go = (b * 16 + kb) // 4, ((b * 16 + kb) % 4) * 128
                    first, last = (kb == 4 * qb), (kb == NKB - 1)
                    nc.tensor.matmul(
                        pc[0:64, 0:n], Vg[g][:, go:go + 64], at0[:, 0:n],
                        start=first, stop=last,
                        tile_position=(0, 0), skip_group_check=True,
                    )
                    nc.tensor.matmul(
                        pc[64:128, 0:n], Vg[g][:, go + 64:go + 128], at1[:, 0:n],
                        start=first, stop=last,
                        tile_position=(0, 64), skip_group_check=True,
                    )

                # ---- attention main loop, software-pipelined ----
                for b in range(B):
                    pend = None                       # score pair awaiting exp+ctx
                    pc = None
                    drip_delay = 10                   # let the A2A land first
                    for qb in range(NQB):
                        for kb in range(4 * qb, NKB):
                            if pend is None:          # prologue of this batch
                                pend = (qb, kb) + emit_score(b, qb, kb)
                                pc = pcx.tile([128, QB], FP, tag="pc",
                                              name=f"pc{b}_{qb}")
                                continue
                            pqb, pkb, ps0, ps1, j, n = pend
                            at0, at1 = emit_exp(b, pqb, pkb, ps0, ps1, j, n)
                            # next score pair ahead of this ctx pair
                            pend = (qb, kb) + emit_score(b, qb, kb)
                            if qb != pqb:             # new q row -> new psum
                                pc_next = pcx.tile([128, QB], FP, tag="pc",
                                                   name=f"pc{b}_{qb}")
                            emit_ctx(b, pqb, pkb, pc, at0, at1, n)
                            if qb != pqb:
                                nc.vector.tensor_copy(
                                    ctxT[b][:, pqb * QB:(pqb + 1) * QB], pc[:]
                                )
                                pc = pc_next
                                if pqb == 1:          # first token-half done
                                    fire_a2a(b, 0)
                            # drip batch-0 out-proj into batch-1's stream
                            if drip_delay > 0:
                                drip_delay -= 1
                            else:
                                for _ in range(5):
                                    emit_outproj_mm()
                    # drain the last pending iteration
                    pqb, pkb, ps0, ps1, j, n = pend
                    at0, at1 = emit_exp(b, pqb, pkb, ps0, ps1, j, n)
                    emit_ctx(b, pqb, pkb, pc, at0, at1, n)
                    nc.vector.tensor_copy(
                        ctxT[b][:, pqb * QB:(pqb + 1) * QB], pc[:]
                    )
                    fire_a2a(b, 1)
                    if dbg:
                        nc.sync.dma_start(
                            dbg_ctxT[b * 128:(b + 1) * 128, :], ctxT[b][:]
                        )
                    # queue this batch's out-projection micro-ops; b=0's are
                    # dripped into b=1's attention, b=1's drain at the tail.
                    for h in range(2):
                        for fo in range(8):
                            for k in range(8):
                                oq.append((b, h, fo, k))

                # tail: whatever out-projection work is still queued
                while oq:
                    emit_outproj_mm()

    nc.compile()
    return nc


def kernel(**inputs):
    x = np.asarray(inputs["x"], np.float32)
    Wq = np.asarray(inputs["Wq"], np.float32)
    Wk = np.asarray(inputs["Wk"], np.float32)
    Wv = np.asarray(inputs["Wv"], np.float32)
    Wo = np.asarray(inputs["Wo"], np.float32)
    bo = np.asarray(inputs["bo"], np.float32)
    # bq/bk/bv are zeros by the problem's input spec; dif = x @ (Wq - Wk)
    # and v = x @ Wv absorb them exactly when zero.

    if "nc" not in _cached:
        _cached["nc"] = _build()
    nc = _cached["nc"]

    xT = np.ascontiguousarray(x.reshape(T, H).T).astype(np.float16)
    Wob = Wo.astype(ml_dtypes.bfloat16)
    maskbf = np.tril(np.ones((128, 128), np.float32), -1).astype(ml_dtypes.bfloat16)

    in_maps = []
    for c in range(NC):
        cols = slice(c * 128, (c + 1) * 128)
        in_maps.append({
            "xT": xT,
            "Wqc": np.ascontiguousarray(Wq[:, cols]),
            "Wkc": np.ascontiguousarray(Wk[:, cols]),
            "Wvc": np.ascontiguousarray(Wv[:, cols]).astype(np.float16),
            "Wob": Wob,
            "bo": bo,
            "maskbf": maskbf,
        })

    res = run_bass_kernel_spmd(nc, in_maps, core_ids=list(range(NC)))

    out = np.empty((B, S, H), np.float32)
    for c in range(NC):
        oT = res.results[c]["out"]                    # [H, 512]
        for b in range(B):
            for h in range(2):
                out[b, h * 1024 + c * OSLOT:h * 1024 + (c + 1) * OSLOT, :] = (
                    oT[:, (2 * b + h) * OSLOT:(2 * b + h + 1) * OSLOT].T
                )
    return out


# revision 13
# speedup vs baseline: 1.0305x; 1.0305x over previous
"""Distributed gaussian-mask attention for trn2 (8 NeuronCores, SPMD).

Problem: B=2, S=2048, H=1024, 16 heads, hd=64.
  q/k/v = x@W*, dif = q - k, score = exp(-0.5 * dif @ dif^T),
  prob = score * triu(ones,k=1), ctx = prob @ v, out = ctx @ Wo + bo.
  (bq/bk/bv are zeros by construction -- folded out; dif = x @ (Wq-Wk).)

Sharding (uniform SPMD program, data-only per-core differences):
  - Head parallel: core c owns heads (2c, 2c+1) = 128 feature columns of
    Wq/Wk/Wv.  Each core computes D^T = (Wq-Wk)c^T-proj and V for ALL
    tokens of its 2 heads, runs the full (anti-)causal attention
    triangle locally (no collective), producing ctx^T [128, 4096].
  - FOUR AllToAlls (one per batch-half, 256 KB each) re-shard ctx from
    head-major to token-major as soon as each half batch of ctx^T is
    ready, overlapping the collectives with attention compute.  Core c
    ends with full-H ctx^T for tokens {b, h*1024 + [128c, 128c+128)};
    it then runs its 1/8 of the output projection with the full Wo.

Perf structure (v2):
  - QB=512 query blocks (4 per batch) -> N=512 matmuls, fewer per-inst
    overheads.
  - dT stored fp16: score matmuls run at 1 col/cycle (vs 2 for f32r)
    and get FWL on the weight load.
  - The two heads' score MMs run CONCURRENTLY on the PE via row-group
    tiling (K=64 each at row offsets 0/64); the two ctx MMs run
    concurrently via col-group tiling (M=64 at col offsets 0/64).
  - exp is split across engines: head 0 on ACT (spline Exp), head 1 on
    DVE via the Schraudolph bitcast trick:
      exp(y) ~ bitcast_f32(int32(y * 2^23/ln2 + (127*2^23 - 366000)))
    (max ~3% sawtooth; end-to-end rel-err simulated at 3.7e-3).
  - Software pipelining: score pair of iteration i+1 is emitted before
    the ctx pair of iteration i so the PE never waits on the exp.
  - batch-0's output projection is drip-fed (4 MMs per attention
    iteration) into the batch-1 attention stream; only batch-1's
    out-projection (plus its half-2 AllToAll) remains on the tail.

Precision: x/Wd/Wv fp16, dT fp16, score PSUM fp32, prob bf16, V bf16,
  ctx PSUM fp32, ctx bf16, Wo bf16, out fp32.
"""
import numpy as np
import ml_dtypes

import concourse.bass as bass
import concourse.bacc as bacc
import concourse.mybir as mybir
import concourse.tile as tile
from concourse.bass_utils import run_bass_kernel_spmd

FP = mybir.dt.float32
F16 = mybir.dt.float16
BF = mybir.dt.bfloat16
I32 = mybir.dt.int32
AF = mybir.ActivationFunctionType
ALU = mybir.AluOpType

NC = 8
B, S, H, NH, HD = 2, 2048, 1024, 16, 64
T = B * S            # 4096 tokens
QB = 512             # query block
KB = 128             # key block
NQB = S // QB        # 4 query blocks per batch
NKB = S // KB        # 16 key blocks per batch
OSLOT = 128          # out-projection tokens per core per (batch, half)

# Two-phase Schraudolph fast-exp: exp(-0.5*x) ~ g1 + 0.704*g2 with
#   g_i = bitcast_f32(int32(A*x + B_i)); the 0.5 averaging weight is
#   folded into B (exponent -1), the second phase is offset half a
#   mantissa period.  Max rel err 0.76% (vs 3.0% single-phase).
EXP_A = float(np.float32(-0.5 * (1 << 23) / np.log(2)))
EXP_B1 = float(np.float32(127 * (1 << 23) - (1 << 23) - 426000.0))
EXP_B2 = float(np.float32(127 * (1 << 23) - (1 << 23) + (1 << 22) - 426000.0))
EXP_W2 = 0.704
ACUT = 832           # ACT exps fused cols [0:ACUT); DVE fast-exps the rest

_cached = {}


def _build(dbg=False):
    nc = bacc.Bacc("TRN2", target_bir_lowering=False, debug=False, num_devices=NC)

    xT = nc.dram_tensor("xT", [H, T], F16, kind="ExternalInput")
    Wqc = nc.dram_tensor("Wqc", [H, 128], FP, kind="ExternalInput")
    Wkc = nc.dram_tensor("Wkc", [H, 128], FP, kind="ExternalInput")
    Wvc = nc.dram_tensor("Wvc", [H, 128], F16, kind="ExternalInput")
    Wob = nc.dram_tensor("Wob", [H, H], BF, kind="ExternalInput")
    bo_d = nc.dram_tensor("bo", [H], FP, kind="ExternalInput")
    mask_d = nc.dram_tensor("maskbf", [128, 128], BF, kind="ExternalInput")
    out_d = nc.dram_tensor("out", [H, 4 * OSLOT], FP, kind="ExternalOutput")
    if dbg:
        dbg_dT = nc.dram_tensor("dbg_dT", [8 * 128, 512], F16, kind="ExternalOutput")
        dbg_V = nc.dram_tensor("dbg_V", [8 * 128, 512], BF, kind="ExternalOutput")
        dbg_ctxT = nc.dram_tensor(
            "dbg_ctxT", [B * 128, 2048], BF, kind="ExternalOutput"
        )

    with tile.TileContext(nc) as tc:
        with (
            tc.tile_pool(name="res", bufs=1) as res,      # resident SBUF
            tc.tile_pool(name="stream", bufs=3) as strm,  # streamed SBUF
            tc.tile_pool(name="dram", bufs=1, space="DRAM") as dram,
        ):
            # ---------------- constants / weights in ----------------
            mask_t = res.tile([128, 128], BF, tag="mask")
            nc.sync.dma_start(mask_t[:], mask_d[:])
            bo_t = res.tile([128, 8], FP, tag="bo")
            nc.sync.dma_start(bo_t[:], bo_d[:].rearrange("(f p) -> p f", p=128))

            wd = []
            wv = []
            wo = []
            for k in range(8):
                wq_k = strm.tile([128, 128], FP, tag="wqk", name=f"wq{k}")
                wk_k = strm.tile([128, 128], FP, tag="wkk", name=f"wk{k}")
                nc.sync.dma_start(wq_k[:], Wqc[k * 128:(k + 1) * 128, :])
                nc.scalar.dma_start(wk_k[:], Wkc[k * 128:(k + 1) * 128, :])
                wd_k = res.tile([128, 128], F16, tag=f"wd{k}", name=f"wd{k}")
                nc.vector.tensor_sub(wd_k[:], wq_k[:], wk_k[:])
                wd.append(wd_k)
                wv_k = res.tile([128, 128], F16, tag=f"wv{k}", name=f"wv{k}")
                nc.scalar.dma_start(wv_k[:], Wvc[k * 128:(k + 1) * 128, :])
                wv.append(wv_k)

            # resident outputs of the projections
            dT = [res.tile([128, 512], F16, tag=f"dT{i}", name=f"dT{i}")
                  for i in range(8)]                  # D^T  [128 feat, 4096 tok]
            Vg = [res.tile([128, 512], BF, tag=f"Vg{i}", name=f"Vg{i}")
                  for i in range(8)]                  # V    [tok, feat] 4 tiles/grp

            # ---------------- projections: D^T and V ----------------
            with tc.tile_pool(name="psp", bufs=1, space="PSUM") as psp:
                for half in range(2):                 # token halves (2048 each)
                    xk_tiles = []
                    for k in range(8):
                        xk = strm.tile([128, 2048], F16, tag="xk", name=f"xk{half}_{k}")
                        # alternate DMA-issuing engines to use both HWDGE rings
                        eng = nc.sync if k % 2 == 0 else nc.scalar
                        eng.dma_start(
                            xk[:], xT[k * 128:(k + 1) * 128,
                                      half * 2048:(half + 1) * 2048]
                        )
                        xk_tiles.append(xk)
                    pd = [psp.tile([128, 512], FP, tag=f"pd{j}", name=f"pd{half}_{j}")
                          for j in range(4)]
                    pv = [psp.tile([128, 512], FP, tag=f"pv{j}", name=f"pv{half}_{j}")
                          for j in range(4)]
                    for k in range(8):
                        xk = xk_tiles[k]
                        for j in range(4):            # 512-token chunks -> D^T
                            nc.tensor.matmul(
                                pd[j][:], wd[k][:], xk[:, j * 512:(j + 1) * 512],
                                start=(k == 0), stop=(k == 7),
                            )
                        for t in range(16):           # 128-token tiles -> V
                            nc.tensor.matmul(
                                pv[t // 4][:, (t % 4) * 128:(t % 4 + 1) * 128],
                                xk[:, t * 128:(t + 1) * 128], wv[k][:],
                                start=(k == 0 and t % 4 == 0), stop=(k == 7),
                                skip_group_check=True,
                            )
                    for j in range(4):
                        nc.vector.tensor_copy(dT[half * 4 + j][:], pd[j][:])
                        nc.vector.tensor_copy(Vg[half * 4 + j][:], pv[j][:])
                if dbg:
                    for i in range(8):
                        nc.sync.dma_start(
                            dbg_dT[i * 128:(i + 1) * 128, :], dT[i][:]
                        )
                        nc.sync.dma_start(
                            dbg_V[i * 128:(i + 1) * 128, :], Vg[i][:]
                        )

            # Wo loads deferred past the projection phase so they don't
            # starve the x-tile DMAs at startup (first use is mid-attention).
            for k in range(8):
                wo_k = res.tile([128, 1024], BF, tag=f"wo{k}", name=f"wo{k}")
                nc.sync.dma_start(wo_k[:], Wob[k * 128:(k + 1) * 128, :])
                wo.append(wo_k)

            # ---------------- attention (local, 2 heads) ----------------
            ctxT = [res.tile([128, 2048], BF, tag=f"ctxT{b}", name=f"ctxT{b}")
                    for b in range(B)]
            # received full-H ctx tiles: [b][half] -> 8 x [128, 128]
            ctxg = [[None, None], [None, None]]

            with (
                tc.tile_pool(name="pss", bufs=2, space="PSUM") as pss,
                tc.tile_pool(name="pcx", bufs=2, space="PSUM") as pcx,
                tc.tile_pool(name="pso", bufs=2, space="PSUM") as pso,
            ):
                # ---- AllToAll for one (batch, token-half): 256 KB ----
                def fire_a2a(b, h):
                    cc_in = dram.tile([1024, OSLOT], BF, name=f"cc_in{b}_{h}")
                    cc_out = dram.tile([1024, OSLOT], BF, name=f"cc_out{b}_{h}")
                    for j in range(8):
                        nc.sync.dma_start(
                            cc_in[j * 128:(j + 1) * 128, :],
                            ctxT[b][:, h * 1024 + j * OSLOT:
                                      h * 1024 + (j + 1) * OSLOT],
                        )
                    nc.gpsimd.collective_compute(
                        "AllToAll",
                        mybir.AluOpType.bypass,
                        replica_groups=[list(range(NC))],
                        ins=[cc_in[:].opt()],
                        outs=[cc_out[:].opt()],
                    )
                    gs = []
                    for k in range(8):
                        g = res.tile([128, OSLOT], BF, tag=f"cg{b}_{h}_{k}",
                                     name=f"cg{b}_{h}_{k}")
                        nc.sync.dma_start(g[:], cc_out[k * 128:(k + 1) * 128, :])
                        gs.append(g)
                    ctxg[b][h] = gs

                # ---- out-projection, drip-fed one MM at a time ----
                oq = []          # pending (b, h, fo, k) micro-ops
                ostate = {}

                def emit_outproj_mm():
                    if not oq:
                        return
                    b, h, fo, k = oq.pop(0)
                    if k == 0:
                        ostate["po"] = pso.tile([128, OSLOT], FP, tag="po",
                                                name=f"po{b}_{h}_{fo}")
                    po = ostate["po"]
                    nc.tensor.matmul(
                        po[:], wo[k][:, fo * 128:(fo + 1) * 128],
                        ctxg[b][h][k][:],
                        start=(k == 0), stop=(k == 7),
                        skip_group_check=True,
                    )
                    if k == 7:
                        ot = strm.tile([128, OSLOT], FP, tag="ot", bufs=4,
                                       name=f"ot{b}_{h}_{fo}")
                        nc.vector.tensor_scalar_add(
                            ot[:], po[:], bo_t[:, fo:fo + 1]
                        )
                        nc.sync.dma_start(
                            out_d[fo * 128:(fo + 1) * 128,
                                  (2 * b + h) * OSLOT:(2 * b + h + 1) * OSLOT],
                            ot[:],
                        )

                # ---- score pair (both heads, concurrent row tiles) ----
                # One fused PSUM tile [128, 1024]: head0 scores in cols
                # [0:512), head1 in [512:1024) -- lets a single ACT
                # instruction exp both heads in one pass.
                def emit_score(b, qb, kb):
                    qt = b * 4 + qb                   # dT tile of this q block
                    koff = b * S + kb * KB
                    kt, kc = koff // 512, koff % 512
                    j = kb - 4 * qb
                    n = 128 * (j + 1) if j < 4 else QB
                    ps = pss.tile([128, 2 * QB], FP, tag="ps",
                                  name=f"ps_{b}_{qb}_{kb}")
                    nc.tensor.matmul(
                        ps[:, 0:n], dT[kt][0:64, kc:kc + 128],
                        dT[qt][0:64, 0:n], start=True, stop=True,
                    )
                    nc.tensor.matmul(
                        ps[:, QB:QB + n], dT[kt][64:128, kc:kc + 128],
                        dT[qt][64:128, 0:n], start=True, stop=True,
                        skip_group_check=True,
                    )
                    return ps, j, n

                # ---- exp: one big ACT instr; DVE two-phase fast-exp tail ----
                def emit_exp(b, qb, kb, ps, j, n):
                    at = strm.tile([128, 2 * QB], BF, tag="at", bufs=3,
                                   name=f"at_{b}_{qb}_{kb}")
                    cut = min(QB + n, ACUT)
                    # ACT: exp over h0 cols [0:n), junk gap [n:512), h1 cols
                    # [0:cut-512).  Junk stays finite and is never consumed.
                    nc.scalar.activation(at[:, 0:cut], ps[:, 0:cut], AF.Exp,
                                         scale=-0.5)
                    if QB + n > ACUT:                 # DVE tail, 2-phase
                        i1 = strm.tile([128, 2 * QB - ACUT], I32, tag="i1",
                                       bufs=2, name=f"i1_{b}_{qb}_{kb}")
                        i2 = strm.tile([128, 2 * QB - ACUT], I32, tag="i2",
                                       bufs=2, name=f"i2_{b}_{qb}_{kb}")
                        w = QB + n - ACUT
                        nc.vector.tensor_scalar(
                            i1[:, 0:w], ps[:, ACUT:QB + n], EXP_A, EXP_B1,
                            ALU.mult, ALU.add,
                        )
                        nc.vector.tensor_scalar(
                            i2[:, 0:w], ps[:, ACUT:QB + n], EXP_A, EXP_B2,
                            ALU.mult, ALU.add,
                        )
                        nc.vector.scalar_tensor_tensor(
                            at[:, ACUT:QB + n], i2[:, 0:w].bitcast(FP), EXP_W2,
                            i1[:, 0:w].bitcast(FP), ALU.mult, ALU.add,
                        )
                    if j < 4:                         # diagonal: mask last 128
                        nc.vector.tensor_mul(
                            at[:, j * 128:n], at[:, j * 128:n], mask_t[:]
                        )
                        nc.vector.tensor_mul(
                            at[:, QB + j * 128:QB + n],
                            at[:, QB + j * 128:QB + n], mask_t[:]
                        )
                    return at

                # ---- ctx pair (both heads, concurrent col tiles) ----
                def emit_ctx(b, qb, kb, pc, at, n):
                    g, go = (b * 16 + kb) // 4, ((b * 16 + kb) % 4) * 128
                    first, last = (kb == 4 * qb), (kb == NKB - 1)
                    nc.tensor.matmul(
                        pc[0:64, 0:n], Vg[g][:, go:go + 64], at[:, 0:n],
                        start=first, stop=last,
                        tile_position=(0, 0), skip_group_check=True,
                    )
                    nc.tensor.matmul(
                        pc[64:128, 0:n], Vg[g][:, go + 64:go + 128],
                        at[:, QB:QB + n],
                        start=first, stop=last,
                        tile_position=(0, 64), skip_group_check=True,
                    )

                # ---- attention main loop, software-pipelined ----
                for b in range(B):
                    pend = None                       # score pair awaiting exp+ctx
                    pc = None
                    drip_delay = 10                   # let the A2A land first
                    for qb in range(NQB):
                        for kb in range(4 * qb, NKB):
                            if pend is None:          # prologue of this batch
                                pend = (qb, kb) + emit_score(b, qb, kb)
                                pc = pcx.tile([128, QB], FP, tag="pc",
                                              name=f"pc{b}_{qb}")
                                continue
                            pqb, pkb, ps, j, n = pend
                            at = emit_exp(b, pqb, pkb, ps, j, n)
                            # next score pair ahead of this ctx pair
                            pend = (qb, kb) + emit_score(b, qb, kb)
                            if qb != pqb:             # new q row -> new psum
                                pc_next = pcx.tile([128, QB], FP, tag="pc",
                                                   name=f"pc{b}_{qb}")
                            emit_ctx(b, pqb, pkb, pc, at, n)
                            if qb != pqb:
                                nc.vector.tensor_copy(
                                    ctxT[b][:, pqb * QB:(pqb + 1) * QB], pc[:]
                                )
                                pc = pc_next
                                if pqb == 1:          # first token-half done
                                    fire_a2a(b, 0)
                            # drip batch-0 out-proj into batch-1's stream
                            if drip_delay > 0:
                                drip_delay -= 1
                            else:
                                for _ in range(5):
                                    emit_outproj_mm()
                    # drain the last pending iteration
                    pqb, pkb, ps, j, n = pend
                    at = emit_exp(b, pqb, pkb, ps, j, n)
                    emit_ctx(b, pqb, pkb, pc, at, n)
                    nc.vector.tensor_copy(
                        ctxT[b][:, pqb * QB:(pqb + 1) * QB], pc[:]
                    )
                    fire_a2a(b, 1)
                    if dbg:
                        nc.sync.dma_start(
                            dbg_ctxT[b * 128:(b + 1) * 128, :], ctxT[b][:]
                        )
                    # queue this batch's out-projection micro-ops; b=0's are
                    # dripped into b=1's attention, b=1's drain at the tail.
                    for h in range(2):
                        for fo in range(8):
                            for k in range(8):
                                oq.append((b, h, fo, k))

                # tail: whatever out-projection work is still queued
                while oq:
                    emit_outproj_mm()

    nc.compile()
    return nc


def kernel(**inputs):
    x = np.asarray(inputs["x"], np.float32)
    Wq = np.asarray(inputs["Wq"], np.float32)
    Wk = np.asarray(inputs["Wk"], np.float32)
    Wv = np.asarray(inputs["Wv"], np.float32)
    Wo = np.asarray(inputs["Wo"], np.float32)
    bo = np.asarray(inputs["bo"], np.float32)
    # bq/bk/bv are zeros by the problem's input spec; dif = x @ (Wq - Wk)
    # and v = x @ Wv absorb them exactly when zero.

    if "nc" not in _cached:
        _cached["nc"] = _build()
    nc = _cached["nc"]

    xT = np.ascontiguousarray(x.reshape(T, H).T).astype(np.float16)
    Wob = Wo.astype(ml_dtypes.bfloat16)
    maskbf = np.tril(np.ones((128, 128), np.float32), -1).astype(ml_dtypes.bfloat16)

    in_maps = []
    for c in range(NC):
        cols = slice(c * 128, (c + 1) * 128)
        in_maps.append({
            "xT": xT,
            "Wqc": np.ascontiguousarray(Wq[:, cols]),
            "Wkc": np.ascontiguousarray(Wk[:, cols]),
            "Wvc": np.ascontiguousarray(Wv[:, cols]).astype(np.float16),
            "Wob": Wob,
            "bo": bo,
            "maskbf": maskbf,
        })

    res = run_bass_kernel_spmd(nc, in_maps, core_ids=list(range(NC)))

    out = np.empty((B, S, H), np.float32)
    for c in range(NC):
        oT = res.results[c]["out"]                    # [H, 512]
        for b in range(B):
            for h in range(2):
                out[b, h * 1024 + c * OSLOT:h * 1024 + (c + 1) * OSLOT, :] = (
                    oT[:, (2 * b + h) * OSLOT:(2 * b + h + 1) * OSLOT].T
                )
    return out


# revision 19
# speedup vs baseline: 1.0633x; 1.0318x over previous
"""Distributed gaussian-mask attention for trn2 (8 NeuronCores, SPMD).

Problem: B=2, S=2048, H=1024, 16 heads, hd=64.
  q/k/v = x@W*, dif = q - k, score = exp(-0.5 * dif @ dif^T),
  prob = score * triu(ones,k=1), ctx = prob @ v, out = ctx @ Wo + bo.
  (bq/bk/bv are zeros by construction -- folded out; dif = x @ (Wq-Wk).)

Sharding (uniform SPMD program, data-only per-core differences):
  - Head parallel: core c owns heads (2c, 2c+1) = 128 feature columns of
    Wq/Wk/Wv.  Each core computes D^T = (Wq-Wk)c^T-proj and V for ALL
    tokens of its 2 heads, runs the full (anti-)causal attention
    triangle locally (no collective), producing ctx^T [128, 4096].
  - FOUR AllToAlls (one per batch-half, 256 KB each) re-shard ctx from
    head-major to token-major as soon as each half batch of ctx^T is
    ready, overlapping the collectives with attention compute.  Core c
    ends with full-H ctx^T for tokens {b, h*1024 + [128c, 128c+128)};
    it then runs its 1/8 of the output projection with the full Wo.

Perf structure (v2):
  - QB=512 query blocks (4 per batch) -> N=512 matmuls, fewer per-inst
    overheads.
  - dT stored fp16: score matmuls run at 1 col/cycle (vs 2 for f32r)
    and get FWL on the weight load.
  - The two heads' score MMs run CONCURRENTLY on the PE via row-group
    tiling (K=64 each at row offsets 0/64); the two ctx MMs run
    concurrently via col-group tiling (M=64 at col offsets 0/64).
  - exp is split across engines: head 0 on ACT (spline Exp), head 1 on
    DVE via the Schraudolph bitcast trick:
      exp(y) ~ bitcast_f32(int32(y * 2^23/ln2 + (127*2^23 - 366000)))
    (max ~3% sawtooth; end-to-end rel-err simulated at 3.7e-3).
  - Software pipelining: score pair of iteration i+1 is emitted before
    the ctx pair of iteration i so the PE never waits on the exp.
  - batch-0's output projection is drip-fed (4 MMs per attention
    iteration) into the batch-1 attention stream; only batch-1's
    out-projection (plus its half-2 AllToAll) remains on the tail.

Precision: x/Wd/Wv fp16, dT fp16, score PSUM fp32, prob bf16, V bf16,
  ctx PSUM fp32, ctx bf16, Wo bf16, out fp32.
"""
import numpy as np
import ml_dtypes

import concourse.bass as bass
import concourse.bacc as bacc
import concourse.mybir as mybir
import concourse.tile as tile
from concourse.bass_utils import run_bass_kernel_spmd

FP = mybir.dt.float32
F16 = mybir.dt.float16
BF = mybir.dt.bfloat16
I32 = mybir.dt.int32
AF = mybir.ActivationFunctionType
ALU = mybir.AluOpType

NC = 8
B, S, H, NH, HD = 2, 2048, 1024, 16, 64
T = B * S            # 4096 tokens
QB = 512             # query block
KB = 128             # key block
NQB = S // QB        # 4 query blocks per batch
NKB = S // KB        # 16 key blocks per batch
OSLOT = 128          # out-projection tokens per core per (batch, half)

# Two-phase Schraudolph fast-exp: exp(-0.5*x) ~ g1 + 0.704*g2 with
#   g_i = bitcast_f32(int32(A*x + B_i)); the 0.5 averaging weight is
#   folded into B (exponent -1), the second phase is offset half a
#   mantissa period.  Max rel err 0.76% (vs 3.0% single-phase).
EXP_A = float(np.float32(-0.5 * (1 << 23) / np.log(2)))
EXP_B1 = float(np.float32(127 * (1 << 23) - (1 << 23) - 426000.0))
EXP_B2 = float(np.float32(127 * (1 << 23) - (1 << 23) + (1 << 22) - 426000.0))
EXP_W2 = 0.704
ACUT = 832           # ACT exps fused cols [0:ACUT); DVE fast-exps the rest

_cached = {}


def _build(dbg=False):
    nc = bacc.Bacc("TRN2", target_bir_lowering=False, debug=False, num_devices=NC)

    xT = nc.dram_tensor("xT", [H, T], F16, kind="ExternalInput")
    Wqc = nc.dram_tensor("Wqc", [H, 128], FP, kind="ExternalInput")
    Wkc = nc.dram_tensor("Wkc", [H, 128], FP, kind="ExternalInput")
    Wvc = nc.dram_tensor("Wvc", [H, 128], F16, kind="ExternalInput")
    Wob = nc.dram_tensor("Wob", [H, H], BF, kind="ExternalInput")
    bo_d = nc.dram_tensor("bo", [H], FP, kind="ExternalInput")
    mask_d = nc.dram_tensor("maskbf", [128, 128], BF, kind="ExternalInput")
    out_d = nc.dram_tensor("out", [H, 4 * OSLOT], FP, kind="ExternalOutput")
    if dbg:
        dbg_dT = nc.dram_tensor("dbg_dT", [8 * 128, 512], F16, kind="ExternalOutput")
        dbg_V = nc.dram_tensor("dbg_V", [8 * 128, 512], BF, kind="ExternalOutput")
        dbg_ctxT = nc.dram_tensor(
            "dbg_ctxT", [B * 128, 2048], BF, kind="ExternalOutput"
        )

    with tile.TileContext(nc) as tc:
        with (
            tc.tile_pool(name="res", bufs=1) as res,      # resident SBUF
            tc.tile_pool(name="stream", bufs=3) as strm,  # streamed SBUF
            tc.tile_pool(name="dram", bufs=1, space="DRAM") as dram,
        ):
            # ---------------- constants / weights in ----------------
            mask_t = res.tile([128, 128], BF, tag="mask")
            nc.sync.dma_start(mask_t[:], mask_d[:])
            bo_t = res.tile([128, 8], FP, tag="bo")
            nc.sync.dma_start(bo_t[:], bo_d[:].rearrange("(f p) -> p f", p=128))

            wd = []
            wv = []
            wo = []
            for k in range(8):
                wq_k = strm.tile([128, 128], FP, tag="wqk", name=f"wq{k}")
                wk_k = strm.tile([128, 128], FP, tag="wkk", name=f"wk{k}")
                nc.sync.dma_start(wq_k[:], Wqc[k * 128:(k + 1) * 128, :])
                nc.scalar.dma_start(wk_k[:], Wkc[k * 128:(k + 1) * 128, :])
                wd_k = res.tile([128, 128], F16, tag=f"wd{k}", name=f"wd{k}")
                nc.vector.tensor_sub(wd_k[:], wq_k[:], wk_k[:])
                wd.append(wd_k)
                wv_k = res.tile([128, 128], F16, tag=f"wv{k}", name=f"wv{k}")
                nc.scalar.dma_start(wv_k[:], Wvc[k * 128:(k + 1) * 128, :])
                wv.append(wv_k)

            # resident outputs of the projections
            dT = [res.tile([128, 512], F16, tag=f"dT{i}", name=f"dT{i}")
                  for i in range(8)]                  # D^T  [128 feat, 4096 tok]
            Vg = [res.tile([128, 512], BF, tag=f"Vg{i}", name=f"Vg{i}")
                  for i in range(8)]                  # V    [tok, feat] 4 tiles/grp

            # ---------------- projections: D^T (both) + V (batch 0) -------
            # batch-1's V projection is NOT done here: its matmuls are
            # drip-fed into batch-0's attention stream (keeps the PE dense
            # enough for the HAM clock-gate to release, and shortens this
            # DMA-bound phase).  Its x tiles stay resident.
            xk1 = []
            with tc.tile_pool(name="psp", bufs=1, space="PSUM") as psp:
                for half in range(2):                 # token halves (2048 each)
                    xk_tiles = []
                    for k in range(8):
                        if half == 0:
                            xk = strm.tile([128, 2048], F16, tag="xk",
                                           name=f"xk{half}_{k}")
                        else:
                            xk = res.tile([128, 2048], F16, tag=f"xk1_{k}",
                                          name=f"xk1_{k}")
                            xk1.append(xk)
                        # alternate DMA-issuing engines to use both HWDGE rings
                        eng = nc.sync if k % 2 == 0 else nc.scalar
                        eng.dma_start(
                            xk[:], xT[k * 128:(k + 1) * 128,
                                      half * 2048:(half + 1) * 2048]
                        )
                        xk_tiles.append(xk)
                    pd = [psp.tile([128, 512], FP, tag=f"pd{j}", name=f"pd{half}_{j}")
                          for j in range(4)]
                    pv = [psp.tile([128, 512], FP, tag=f"pv{j}", name=f"pv{half}_{j}")
                          for j in range(4)]
                    for k in range(8):
                        xk = xk_tiles[k]
                        for j in range(4):            # 512-token chunks -> D^T
                            nc.tensor.matmul(
                                pd[j][:], wd[k][:], xk[:, j * 512:(j + 1) * 512],
                                start=(k == 0), stop=(k == 7),
                            )
                        if half == 0:
                            for t in range(16):       # 128-token tiles -> V
                                nc.tensor.matmul(
                                    pv[t // 4][:, (t % 4) * 128:(t % 4 + 1) * 128],
                                    xk[:, t * 128:(t + 1) * 128], wv[k][:],
                                    start=(k == 0 and t % 4 == 0), stop=(k == 7),
                                    skip_group_check=True,
                                )
                    for j in range(4):
                        nc.vector.tensor_copy(dT[half * 4 + j][:], pd[j][:])
                        if half == 0:
                            nc.vector.tensor_copy(Vg[half * 4 + j][:], pv[j][:])
                if dbg:
                    for i in range(8):
                        nc.sync.dma_start(
                            dbg_dT[i * 128:(i + 1) * 128, :], dT[i][:]
                        )
                        nc.sync.dma_start(
                            dbg_V[i * 128:(i + 1) * 128, :], Vg[i][:]
                        )

            # Wo loads deferred past the projection phase so they don't
            # starve the x-tile DMAs at startup (first use is mid-attention).
            for k in range(8):
                wo_k = res.tile([128, 1024], BF, tag=f"wo{k}", name=f"wo{k}")
                nc.sync.dma_start(wo_k[:], Wob[k * 128:(k + 1) * 128, :])
                wo.append(wo_k)

            # ---------------- attention (local, 2 heads) ----------------
            ctxT = [res.tile([128, 2048], BF, tag=f"ctxT{b}", name=f"ctxT{b}")
                    for b in range(B)]
            # received full-H ctx tiles: [b][half] -> 8 x [128, 128]
            ctxg = [[None, None], [None, None]]

            with (
                tc.tile_pool(name="pss", bufs=2, space="PSUM") as pss,
                tc.tile_pool(name="pcx", bufs=1, space="PSUM") as pcx,
                tc.tile_pool(name="pso", bufs=2, space="PSUM") as pso,
                tc.tile_pool(name="pvd", bufs=1, space="PSUM") as pvd,
            ):
                # ---- batch-1 V projection, drip-fed into b0's attention ----
                vq = [(j, k, t) for j in range(4) for k in range(8)
                      for t in range(4 * j, 4 * j + 4)]
                vstate = {}

                def emit_vproj_mm():
                    if not vq:
                        return
                    j, k, t = vq.pop(0)
                    if k == 0 and t == 4 * j:
                        vstate["pv"] = pvd.tile([128, 512], FP, tag="pvd",
                                                name=f"pvd{j}")
                    pv_t = vstate["pv"]
                    nc.tensor.matmul(
                        pv_t[:, (t % 4) * 128:(t % 4 + 1) * 128],
                        xk1[k][:, t * 128:(t + 1) * 128], wv[k][:],
                        start=(k == 0 and t == 4 * j), stop=(k == 7),
                        skip_group_check=True,
                    )
                    if k == 7 and t == 4 * j + 3:
                        nc.vector.tensor_copy(Vg[4 + j][:], pv_t[:])
                # ---- AllToAll for one (batch, token-half): 256 KB ----
                def fire_a2a(b, h):
                    cc_in = dram.tile([1024, OSLOT], BF, name=f"cc_in{b}_{h}")
                    cc_out = dram.tile([1024, OSLOT], BF, name=f"cc_out{b}_{h}")
                    for j in range(8):
                        nc.sync.dma_start(
                            cc_in[j * 128:(j + 1) * 128, :],
                            ctxT[b][:, h * 1024 + j * OSLOT:
                                      h * 1024 + (j + 1) * OSLOT],
                        )
                    nc.gpsimd.collective_compute(
                        "AllToAll",
                        mybir.AluOpType.bypass,
                        replica_groups=[list(range(NC))],
                        ins=[cc_in[:].opt()],
                        outs=[cc_out[:].opt()],
                    )
                    gs = []
                    for k in range(8):
                        g = res.tile([128, OSLOT], BF, tag=f"cg{b}_{h}_{k}",
                                     name=f"cg{b}_{h}_{k}")
                        nc.sync.dma_start(g[:], cc_out[k * 128:(k + 1) * 128, :])
                        gs.append(g)
                    ctxg[b][h] = gs

                # ---- out-projection, drip-fed one MM at a time ----
                oq = []          # pending (b, h, fo, k) micro-ops
                ostate = {}

                def emit_outproj_mm():
                    if not oq:
                        return
                    b, h, fo, k = oq.pop(0)
                    if k == 0:
                        ostate["po"] = pso.tile([128, OSLOT], FP, tag="po",
                                                name=f"po{b}_{h}_{fo}")
                    po = ostate["po"]
                    nc.tensor.matmul(
                        po[:], wo[k][:, fo * 128:(fo + 1) * 128],
                        ctxg[b][h][k][:],
                        start=(k == 0), stop=(k == 7),
                        skip_group_check=True,
                    )
                    if k == 7:
                        ot = strm.tile([128, OSLOT], FP, tag="ot", bufs=4,
                                       name=f"ot{b}_{h}_{fo}")
                        nc.vector.tensor_scalar_add(
                            ot[:], po[:], bo_t[:, fo:fo + 1]
                        )
                        nc.sync.dma_start(
                            out_d[fo * 128:(fo + 1) * 128,
                                  (2 * b + h) * OSLOT:(2 * b + h + 1) * OSLOT],
                            ot[:],
                        )

                # ---- score pair (both heads, concurrent row tiles) ----
                # One fused PSUM tile [128, 1024]: head0 scores in cols
                # [0:512), head1 in [512:1024) -- lets a single ACT
                # instruction exp both heads in one pass.
                def emit_score(b, qb, kb):
                    qt = b * 4 + qb                   # dT tile of this q block
                    koff = b * S + kb * KB
                    kt, kc = koff // 512, koff % 512
                    j = kb - 4 * qb
                    n = 128 * (j + 1) if j < 4 else QB
                    ps = pss.tile([128, 2 * QB], FP, tag="ps",
                                  name=f"ps_{b}_{qb}_{kb}")
                    nc.tensor.matmul(
                        ps[:, 0:n], dT[kt][0:64, kc:kc + 128],
                        dT[qt][0:64, 0:n], start=True, stop=True,
                    )
                    nc.tensor.matmul(
                        ps[:, QB:QB + n], dT[kt][64:128, kc:kc + 128],
                        dT[qt][64:128, 0:n], start=True, stop=True,
                        skip_group_check=True,
                    )
                    return ps, j, n

                # ---- exp: one big ACT instr; DVE two-phase fast-exp tail ----
                def emit_exp(b, qb, kb, ps, j, n):
                    at = strm.tile([128, 2 * QB], BF, tag="at", bufs=3,
                                   name=f"at_{b}_{qb}_{kb}")
                    cut = min(QB + n, ACUT)
                    # ACT: exp over h0 cols [0:n), junk gap [n:512), h1 cols
                    # [0:cut-512).  Junk stays finite and is never consumed.
                    nc.scalar.activation(at[:, 0:cut], ps[:, 0:cut], AF.Exp,
                                         scale=-0.5)
                    if QB + n > ACUT:                 # DVE tail, 2-phase
                        i1 = strm.tile([128, 2 * QB - ACUT], I32, tag="i1",
                                       bufs=2, name=f"i1_{b}_{qb}_{kb}")
                        i2 = strm.tile([128, 2 * QB - ACUT], I32, tag="i2",
                                       bufs=2, name=f"i2_{b}_{qb}_{kb}")
                        w = QB + n - ACUT
                        nc.vector.tensor_scalar(
                            i1[:, 0:w], ps[:, ACUT:QB + n], EXP_A, EXP_B1,
                            ALU.mult, ALU.add,
                        )
                        nc.vector.tensor_scalar(
                            i2[:, 0:w], ps[:, ACUT:QB + n], EXP_A, EXP_B2,
                            ALU.mult, ALU.add,
                        )
                        nc.vector.scalar_tensor_tensor(
                            at[:, ACUT:QB + n], i2[:, 0:w].bitcast(FP), EXP_W2,
                            i1[:, 0:w].bitcast(FP), ALU.mult, ALU.add,
                        )
                    if j < 4:                         # diagonal: mask last 128
                        nc.vector.tensor_mul(
                            at[:, j * 128:n], at[:, j * 128:n], mask_t[:]
                        )
                        nc.vector.tensor_mul(
                            at[:, QB + j * 128:QB + n],
                            at[:, QB + j * 128:QB + n], mask_t[:]
                        )
                    return at

                # ---- ctx pair (both heads, concurrent col tiles) ----
                def emit_ctx(b, qb, kb, pc, at, n):
                    g, go = (b * 16 + kb) // 4, ((b * 16 + kb) % 4) * 128
                    first, last = (kb == 4 * qb), (kb == NKB - 1)
                    nc.tensor.matmul(
                        pc[0:64, 0:n], Vg[g][:, go:go + 64], at[:, 0:n],
                        start=first, stop=last,
                        tile_position=(0, 0), skip_group_check=True,
                    )
                    nc.tensor.matmul(
                        pc[64:128, 0:n], Vg[g][:, go + 64:go + 128],
                        at[:, QB:QB + n],
                        start=first, stop=last,
                        tile_position=(0, 64), skip_group_check=True,
                    )

                # ---- attention main loop, software-pipelined ----
                for b in range(B):
                    pend = None                       # score pair awaiting exp+ctx
                    pc = None
                    drip_delay = 6 if b == 0 else 20  # let DMAs / A2A land first
                    for qb in range(NQB):
                        for kb in range(4 * qb, NKB):
                            if pend is None:          # prologue of this batch
                                pend = (qb, kb) + emit_score(b, qb, kb)
                                pc = pcx.tile([128, QB], FP, tag="pc",
                                              name=f"pc{b}_{qb}")
                                continue
                            pqb, pkb, ps, j, n = pend
                            at = emit_exp(b, pqb, pkb, ps, j, n)
                            # next score pair ahead of this ctx pair
                            pend = (qb, kb) + emit_score(b, qb, kb)
                            if qb != pqb:             # new q row -> new psum
                                pc_next = pcx.tile([128, QB], FP, tag="pc",
                                                   name=f"pc{b}_{qb}")
                            emit_ctx(b, pqb, pkb, pc, at, n)
                            if qb != pqb:
                                nc.vector.tensor_copy(
                                    ctxT[b][:, pqb * QB:(pqb + 1) * QB], pc[:]
                                )
                                pc = pc_next
                                if pqb == 1:          # first token-half done
                                    fire_a2a(b, 0)
                            # drip deferred work into this batch's stream:
                            # b0 gets batch-1's V projection, b1 gets
                            # batch-0's out-projection.
                            if drip_delay > 0:
                                drip_delay -= 1
                            elif b == 0:
                                for _ in range(4):
                                    emit_vproj_mm()
                            else:
                                for _ in range(7):
                                    emit_outproj_mm()
                    # drain the last pending iteration
                    pqb, pkb, ps, j, n = pend
                    at = emit_exp(b, pqb, pkb, ps, j, n)
                    emit_ctx(b, pqb, pkb, pc, at, n)
                    nc.vector.tensor_copy(
                        ctxT[b][:, pqb * QB:(pqb + 1) * QB], pc[:]
                    )
                    fire_a2a(b, 1)
                    if b == 0:
                        # finish any V-projection work before b1's ctx needs it
                        while vq:
                            emit_vproj_mm()
                    if dbg:
                        nc.sync.dma_start(
                            dbg_ctxT[b * 128:(b + 1) * 128, :], ctxT[b][:]
                        )
                    # queue this batch's out-projection micro-ops; b=0's are
                    # dripped into b=1's attention, b=1's drain at the tail.
                    for h in range(2):
                        for fo in range(8):
                            for k in range(8):
                                oq.append((b, h, fo, k))

                # tail: whatever out-projection work is still queued
                while oq:
                    emit_outproj_mm()

    nc.compile()
    return nc


def kernel(**inputs):
    x = np.asarray(inputs["x"], np.float32)
    Wq = np.asarray(inputs["Wq"], np.float32)
    Wk = np.asarray(inputs["Wk"], np.float32)
    Wv = np.asarray(inputs["Wv"], np.float32)
    Wo = np.asarray(inputs["Wo"], np.float32)
    bo = np.asarray(inputs["bo"], np.float32)
    # bq/bk/bv are zeros by the problem's input spec; dif = x @ (Wq - Wk)
    # and v = x @ Wv absorb them exactly when zero.

    if "nc" not in _cached:
        _cached["nc"] = _build()
    nc = _cached["nc"]

    xT = np.ascontiguousarray(x.reshape(T, H).T).astype(np.float16)
    Wob = Wo.astype(ml_dtypes.bfloat16)
    maskbf = np.tril(np.ones((128, 128), np.float32), -1).astype(ml_dtypes.bfloat16)

    in_maps = []
    for c in range(NC):
        cols = slice(c * 128, (c + 1) * 128)
        in_maps.append({
            "xT": xT,
            "Wqc": np.ascontiguousarray(Wq[:, cols]),
            "Wkc": np.ascontiguousarray(Wk[:, cols]),
            "Wvc": np.ascontiguousarray(Wv[:, cols]).astype(np.float16),
            "Wob": Wob,
            "bo": bo,
            "maskbf": maskbf,
        })

    res = run_bass_kernel_spmd(nc, in_maps, core_ids=list(range(NC)))

    out = np.empty((B, S, H), np.float32)
    for c in range(NC):
        oT = res.results[c]["out"]                    # [H, 512]
        for b in range(B):
            for h in range(2):
                out[b, h * 1024 + c * OSLOT:h * 1024 + (c + 1) * OSLOT, :] = (
                    oT[:, (2 * b + h) * OSLOT:(2 * b + h + 1) * OSLOT].T
                )
    return out


# revision 20
# speedup vs baseline: 1.0683x; 1.0047x over previous
"""Distributed gaussian-mask attention for trn2 (8 NeuronCores, SPMD).

Problem: B=2, S=2048, H=1024, 16 heads, hd=64.
  q/k/v = x@W*, dif = q - k, score = exp(-0.5 * dif @ dif^T),
  prob = score * triu(ones,k=1), ctx = prob @ v, out = ctx @ Wo + bo.
  (bq/bk/bv are zeros by construction -- folded out; dif = x @ (Wq-Wk).)

Sharding (uniform SPMD program, data-only per-core differences):
  - Head parallel: core c owns heads (2c, 2c+1) = 128 feature columns of
    Wq/Wk/Wv.  Each core computes D^T = (Wq-Wk)c^T-proj and V for ALL
    tokens of its 2 heads, runs the full (anti-)causal attention
    triangle locally (no collective), producing ctx^T [128, 4096].
  - FOUR AllToAlls (one per batch-half, 256 KB each) re-shard ctx from
    head-major to token-major as soon as each half batch of ctx^T is
    ready, overlapping the collectives with attention compute.  Core c
    ends with full-H ctx^T for tokens {b, h*1024 + [128c, 128c+128)};
    it then runs its 1/8 of the output projection with the full Wo.

Perf structure (v2):
  - QB=512 query blocks (4 per batch) -> N=512 matmuls, fewer per-inst
    overheads.
  - dT stored fp16: score matmuls run at 1 col/cycle (vs 2 for f32r)
    and get FWL on the weight load.
  - The two heads' score MMs run CONCURRENTLY on the PE via row-group
    tiling (K=64 each at row offsets 0/64); the two ctx MMs run
    concurrently via col-group tiling (M=64 at col offsets 0/64).
  - exp is split across engines: head 0 on ACT (spline Exp), head 1 on
    DVE via the Schraudolph bitcast trick:
      exp(y) ~ bitcast_f32(int32(y * 2^23/ln2 + (127*2^23 - 366000)))
    (max ~3% sawtooth; end-to-end rel-err simulated at 3.7e-3).
  - Software pipelining: score pair of iteration i+1 is emitted before
    the ctx pair of iteration i so the PE never waits on the exp.
  - batch-0's output projection is drip-fed (4 MMs per attention
    iteration) into the batch-1 attention stream; only batch-1's
    out-projection (plus its half-2 AllToAll) remains on the tail.

Precision: x/Wd/Wv fp16, dT fp16, score PSUM fp32, prob bf16, V bf16,
  ctx PSUM fp32, ctx bf16, Wo bf16, out fp32.
"""
import numpy as np
import ml_dtypes

import concourse.bass as bass
import concourse.bacc as bacc
import concourse.mybir as mybir
import concourse.tile as tile
from concourse.bass_utils import run_bass_kernel_spmd

FP = mybir.dt.float32
F16 = mybir.dt.float16
BF = mybir.dt.bfloat16
I32 = mybir.dt.int32
AF = mybir.ActivationFunctionType
ALU = mybir.AluOpType

NC = 8
B, S, H, NH, HD = 2, 2048, 1024, 16, 64
T = B * S            # 4096 tokens
QB = 512             # query block
KB = 128             # key block
NQB = S // QB        # 4 query blocks per batch
NKB = S // KB        # 16 key blocks per batch
OSLOT = 128          # out-projection tokens per core per (batch, half)

# Two-phase Schraudolph fast-exp: exp(-0.5*x) ~ g1 + 0.704*g2 with
#   g_i = bitcast_f32(int32(A*x + B_i)); the 0.5 averaging weight is
#   folded into B (exponent -1), the second phase is offset half a
#   mantissa period.  Max rel err 0.76% (vs 3.0% single-phase).
EXP_A = float(np.float32(-0.5 * (1 << 23) / np.log(2)))
EXP_B1 = float(np.float32(127 * (1 << 23) - (1 << 23) - 426000.0))
EXP_B2 = float(np.float32(127 * (1 << 23) - (1 << 23) + (1 << 22) - 426000.0))
EXP_W2 = 0.704
ACUT = 832           # ACT exps fused cols [0:ACUT); DVE fast-exps the rest

_cached = {}


def _build(dbg=False):
    nc = bacc.Bacc("TRN2", target_bir_lowering=False, debug=False, num_devices=NC)

    xT = nc.dram_tensor("xT", [H, T], F16, kind="ExternalInput")
    Wqc = nc.dram_tensor("Wqc", [H, 128], FP, kind="ExternalInput")
    Wkc = nc.dram_tensor("Wkc", [H, 128], FP, kind="ExternalInput")
    Wvc = nc.dram_tensor("Wvc", [H, 128], F16, kind="ExternalInput")
    Wob = nc.dram_tensor("Wob", [H, H], BF, kind="ExternalInput")
    bo_d = nc.dram_tensor("bo", [H], FP, kind="ExternalInput")
    mask_d = nc.dram_tensor("maskbf", [128, 128], BF, kind="ExternalInput")
    out_d = nc.dram_tensor("out", [H, 4 * OSLOT], FP, kind="ExternalOutput")
    if dbg:
        dbg_dT = nc.dram_tensor("dbg_dT", [8 * 128, 512], F16, kind="ExternalOutput")
        dbg_V = nc.dram_tensor("dbg_V", [8 * 128, 512], BF, kind="ExternalOutput")
        dbg_ctxT = nc.dram_tensor(
            "dbg_ctxT", [B * 128, 2048], BF, kind="ExternalOutput"
        )

    with tile.TileContext(nc) as tc:
        with (
            tc.tile_pool(name="res", bufs=1) as res,      # resident SBUF
            tc.tile_pool(name="stream", bufs=3) as strm,  # streamed SBUF
            tc.tile_pool(name="dram", bufs=1, space="DRAM") as dram,
        ):
            # ---------------- constants / weights in ----------------
            mask_t = res.tile([128, 128], BF, tag="mask")
            nc.sync.dma_start(mask_t[:], mask_d[:])
            bo_t = res.tile([128, 8], FP, tag="bo")
            nc.sync.dma_start(bo_t[:], bo_d[:].rearrange("(f p) -> p f", p=128))

            # Tiny AllToAll up front: absorbs cross-core launch skew inside
            # the DMA-bound startup window (gpsimd is idle; compute engines
            # don't block on it).  Without this, the first real AllToAll
            # pays ~25us of rendezvous skew mid-attention and everything
            # queued behind it on gpsimd slips.
            sync_in = dram.tile([128, 8], BF, name="sync_in")
            sync_out = dram.tile([128, 8], BF, name="sync_out")
            nc.sync.dma_start(sync_in[:], mask_d[:, 0:8])
            nc.gpsimd.collective_compute(
                "AllToAll",
                mybir.AluOpType.bypass,
                replica_groups=[list(range(NC))],
                ins=[sync_in[:].opt()],
                outs=[sync_out[:].opt()],
            )

            wd = []
            wv = []
            wo = []
            for k in range(8):
                wq_k = strm.tile([128, 128], FP, tag="wqk", name=f"wq{k}")
                wk_k = strm.tile([128, 128], FP, tag="wkk", name=f"wk{k}")
                nc.sync.dma_start(wq_k[:], Wqc[k * 128:(k + 1) * 128, :])
                nc.scalar.dma_start(wk_k[:], Wkc[k * 128:(k + 1) * 128, :])
                wd_k = res.tile([128, 128], F16, tag=f"wd{k}", name=f"wd{k}")
                nc.vector.tensor_sub(wd_k[:], wq_k[:], wk_k[:])
                wd.append(wd_k)
                wv_k = res.tile([128, 128], F16, tag=f"wv{k}", name=f"wv{k}")
                nc.scalar.dma_start(wv_k[:], Wvc[k * 128:(k + 1) * 128, :])
                wv.append(wv_k)

            # resident outputs of the projections
            dT = [res.tile([128, 512], F16, tag=f"dT{i}", name=f"dT{i}")
                  for i in range(8)]                  # D^T  [128 feat, 4096 tok]
            Vg = [res.tile([128, 512], BF, tag=f"Vg{i}", name=f"Vg{i}")
                  for i in range(8)]                  # V    [tok, feat] 4 tiles/grp

            # ---------------- projections: D^T (both) + V (batch 0) -------
            # batch-1's V projection is NOT done here: its matmuls are
            # drip-fed into batch-0's attention stream (keeps the PE dense
            # enough for the HAM clock-gate to release, and shortens this
            # DMA-bound phase).  Its x tiles stay resident.
            xk1 = []
            with tc.tile_pool(name="psp", bufs=1, space="PSUM") as psp:
                for half in range(2):                 # token halves (2048 each)
                    xk_tiles = []
                    for k in range(8):
                        if half == 0:
                            xk = strm.tile([128, 2048], F16, tag="xk",
                                           name=f"xk{half}_{k}")
                        else:
                            xk = res.tile([128, 2048], F16, tag=f"xk1_{k}",
                                          name=f"xk1_{k}")
                            xk1.append(xk)
                        # alternate DMA-issuing engines to use both HWDGE rings
                        eng = nc.sync if k % 2 == 0 else nc.scalar
                        eng.dma_start(
                            xk[:], xT[k * 128:(k + 1) * 128,
                                      half * 2048:(half + 1) * 2048]
                        )
                        xk_tiles.append(xk)
                    pd = [psp.tile([128, 512], FP, tag=f"pd{j}", name=f"pd{half}_{j}")
                          for j in range(4)]
                    pv = [psp.tile([128, 512], FP, tag=f"pv{j}", name=f"pv{half}_{j}")
                          for j in range(4)]
                    for k in range(8):
                        xk = xk_tiles[k]
                        for j in range(4):            # 512-token chunks -> D^T
                            nc.tensor.matmul(
                                pd[j][:], wd[k][:], xk[:, j * 512:(j + 1) * 512],
                                start=(k == 0), stop=(k == 7),
                            )
                        if half == 0:
                            for t in range(16):       # 128-token tiles -> V
                                nc.tensor.matmul(
                                    pv[t // 4][:, (t % 4) * 128:(t % 4 + 1) * 128],
                                    xk[:, t * 128:(t + 1) * 128], wv[k][:],
                                    start=(k == 0 and t % 4 == 0), stop=(k == 7),
                                    skip_group_check=True,
                                )
                    for j in range(4):
                        nc.vector.tensor_copy(dT[half * 4 + j][:], pd[j][:])
                        if half == 0:
                            nc.vector.tensor_copy(Vg[half * 4 + j][:], pv[j][:])
                if dbg:
                    for i in range(8):
                        nc.sync.dma_start(
                            dbg_dT[i * 128:(i + 1) * 128, :], dT[i][:]
                        )
                        nc.sync.dma_start(
                            dbg_V[i * 128:(i + 1) * 128, :], Vg[i][:]
                        )

            # Wo loads deferred past the projection phase so they don't
            # starve the x-tile DMAs at startup (first use is mid-attention).
            for k in range(8):
                wo_k = res.tile([128, 1024], BF, tag=f"wo{k}", name=f"wo{k}")
                nc.sync.dma_start(wo_k[:], Wob[k * 128:(k + 1) * 128, :])
                wo.append(wo_k)

            # ---------------- attention (local, 2 heads) ----------------
            ctxT = [res.tile([128, 2048], BF, tag=f"ctxT{b}", name=f"ctxT{b}")
                    for b in range(B)]
            # received full-H ctx tiles: [b][half] -> 8 x [128, 128]
            ctxg = [[None, None], [None, None]]

            with (
                tc.tile_pool(name="pss", bufs=2, space="PSUM") as pss,
                tc.tile_pool(name="pcx", bufs=1, space="PSUM") as pcx,
                tc.tile_pool(name="pso", bufs=2, space="PSUM") as pso,
                tc.tile_pool(name="pvd", bufs=1, space="PSUM") as pvd,
            ):
                # ---- batch-1 V projection, drip-fed into b0's attention ----
                vq = [(j, k, t) for j in range(4) for k in range(8)
                      for t in range(4 * j, 4 * j + 4)]
                vstate = {}

                def emit_vproj_mm():
                    if not vq:
                        return
                    j, k, t = vq.pop(0)
                    if k == 0 and t == 4 * j:
                        vstate["pv"] = pvd.tile([128, 512], FP, tag="pvd",
                                                name=f"pvd{j}")
                    pv_t = vstate["pv"]
                    nc.tensor.matmul(
                        pv_t[:, (t % 4) * 128:(t % 4 + 1) * 128],
                        xk1[k][:, t * 128:(t + 1) * 128], wv[k][:],
                        start=(k == 0 and t == 4 * j), stop=(k == 7),
                        skip_group_check=True,
                    )
                    if k == 7 and t == 4 * j + 3:
                        nc.vector.tensor_copy(Vg[4 + j][:], pv_t[:])
                # ---- AllToAll for one (batch, token-half): 256 KB ----
                def fire_a2a(b, h):
                    cc_in = dram.tile([1024, OSLOT], BF, name=f"cc_in{b}_{h}")
                    cc_out = dram.tile([1024, OSLOT], BF, name=f"cc_out{b}_{h}")
                    for j in range(8):
                        nc.sync.dma_start(
                            cc_in[j * 128:(j + 1) * 128, :],
                            ctxT[b][:, h * 1024 + j * OSLOT:
                                      h * 1024 + (j + 1) * OSLOT],
                        )
                    nc.gpsimd.collective_compute(
                        "AllToAll",
                        mybir.AluOpType.bypass,
                        replica_groups=[list(range(NC))],
                        ins=[cc_in[:].opt()],
                        outs=[cc_out[:].opt()],
                    )
                    gs = []
                    for k in range(8):
                        g = res.tile([128, OSLOT], BF, tag=f"cg{b}_{h}_{k}",
                                     name=f"cg{b}_{h}_{k}")
                        nc.sync.dma_start(g[:], cc_out[k * 128:(k + 1) * 128, :])
                        gs.append(g)
                    ctxg[b][h] = gs

                # ---- out-projection, drip-fed one MM at a time ----
                oq = []          # pending (b, h, fo, k) micro-ops
                ostate = {}

                def emit_outproj_mm():
                    if not oq:
                        return
                    b, h, fo, k = oq.pop(0)
                    if k == 0:
                        ostate["po"] = pso.tile([128, OSLOT], FP, tag="po",
                                                name=f"po{b}_{h}_{fo}")
                    po = ostate["po"]
                    nc.tensor.matmul(
                        po[:], wo[k][:, fo * 128:(fo + 1) * 128],
                        ctxg[b][h][k][:],
                        start=(k == 0), stop=(k == 7),
                        skip_group_check=True,
                    )
                    if k == 7:
                        ot = strm.tile([128, OSLOT], FP, tag="ot", bufs=4,
                                       name=f"ot{b}_{h}_{fo}")
                        nc.vector.tensor_scalar_add(
                            ot[:], po[:], bo_t[:, fo:fo + 1]
                        )
                        nc.sync.dma_start(
                            out_d[fo * 128:(fo + 1) * 128,
                                  (2 * b + h) * OSLOT:(2 * b + h + 1) * OSLOT],
                            ot[:],
                        )

                # ---- score pair (both heads, concurrent row tiles) ----
                # One fused PSUM tile [128, 1024]: head0 scores in cols
                # [0:512), head1 in [512:1024) -- lets a single ACT
                # instruction exp both heads in one pass.
                def emit_score(b, qb, kb):
                    qt = b * 4 + qb                   # dT tile of this q block
                    koff = b * S + kb * KB
                    kt, kc = koff // 512, koff % 512
                    j = kb - 4 * qb
                    n = 128 * (j + 1) if j < 4 else QB
                    ps = pss.tile([128, 2 * QB], FP, tag="ps",
                                  name=f"ps_{b}_{qb}_{kb}")
                    nc.tensor.matmul(
                        ps[:, 0:n], dT[kt][0:64, kc:kc + 128],
                        dT[qt][0:64, 0:n], start=True, stop=True,
                    )
                    nc.tensor.matmul(
                        ps[:, QB:QB + n], dT[kt][64:128, kc:kc + 128],
                        dT[qt][64:128, 0:n], start=True, stop=True,
                        skip_group_check=True,
                    )
                    return ps, j, n

                # ---- exp: one big ACT instr; DVE two-phase fast-exp tail ----
                def emit_exp(b, qb, kb, ps, j, n):
                    at = strm.tile([128, 2 * QB], BF, tag="at", bufs=3,
                                   name=f"at_{b}_{qb}_{kb}")
                    cut = min(QB + n, ACUT)
                    # ACT: exp over h0 cols [0:n), junk gap [n:512), h1 cols
                    # [0:cut-512).  Junk stays finite and is never consumed.
                    nc.scalar.activation(at[:, 0:cut], ps[:, 0:cut], AF.Exp,
                                         scale=-0.5)
                    if QB + n > ACUT:                 # DVE tail, 2-phase
                        i1 = strm.tile([128, 2 * QB - ACUT], I32, tag="i1",
                                       bufs=2, name=f"i1_{b}_{qb}_{kb}")
                        i2 = strm.tile([128, 2 * QB - ACUT], I32, tag="i2",
                                       bufs=2, name=f"i2_{b}_{qb}_{kb}")
                        w = QB + n - ACUT
                        nc.vector.tensor_scalar(
                            i1[:, 0:w], ps[:, ACUT:QB + n], EXP_A, EXP_B1,
                            ALU.mult, ALU.add,
                        )
                        nc.vector.tensor_scalar(
                            i2[:, 0:w], ps[:, ACUT:QB + n], EXP_A, EXP_B2,
                            ALU.mult, ALU.add,
                        )
                        nc.vector.scalar_tensor_tensor(
                            at[:, ACUT:QB + n], i2[:, 0:w].bitcast(FP), EXP_W2,
                            i1[:, 0:w].bitcast(FP), ALU.mult, ALU.add,
                        )
                    if j < 4:                         # diagonal: mask last 128
                        nc.vector.tensor_mul(
                            at[:, j * 128:n], at[:, j * 128:n], mask_t[:]
                        )
                        nc.vector.tensor_mul(
                            at[:, QB + j * 128:QB + n],
                            at[:, QB + j * 128:QB + n], mask_t[:]
                        )
                    return at

                # ---- ctx pair (both heads, concurrent col tiles) ----
                def emit_ctx(b, qb, kb, pc, at, n):
                    g, go = (b * 16 + kb) // 4, ((b * 16 + kb) % 4) * 128
                    first, last = (kb == 4 * qb), (kb == NKB - 1)
                    nc.tensor.matmul(
                        pc[0:64, 0:n], Vg[g][:, go:go + 64], at[:, 0:n],
                        start=first, stop=last,
                        tile_position=(0, 0), skip_group_check=True,
                    )
                    nc.tensor.matmul(
                        pc[64:128, 0:n], Vg[g][:, go + 64:go + 128],
                        at[:, QB:QB + n],
                        start=first, stop=last,
                        tile_position=(0, 64), skip_group_check=True,
                    )

                # ---- attention main loop, software-pipelined ----
                for b in range(B):
                    pend = None                       # score pair awaiting exp+ctx
                    pc = None
                    drip_delay = 6 if b == 0 else 20  # let DMAs / A2A land first
                    for qb in range(NQB):
                        for kb in range(4 * qb, NKB):
                            if pend is None:          # prologue of this batch
                                pend = (qb, kb) + emit_score(b, qb, kb)
                                pc = pcx.tile([128, QB], FP, tag="pc",
                                              name=f"pc{b}_{qb}")
                                continue
                            pqb, pkb, ps, j, n = pend
                            at = emit_exp(b, pqb, pkb, ps, j, n)
                            # next score pair ahead of this ctx pair
                            pend = (qb, kb) + emit_score(b, qb, kb)
                            if qb != pqb:             # new q row -> new psum
                                pc_next = pcx.tile([128, QB], FP, tag="pc",
                                                   name=f"pc{b}_{qb}")
                            emit_ctx(b, pqb, pkb, pc, at, n)
                            if qb != pqb:
                                nc.vector.tensor_copy(
                                    ctxT[b][:, pqb * QB:(pqb + 1) * QB], pc[:]
                                )
                                pc = pc_next
                                if pqb == 1:          # first token-half done
                                    fire_a2a(b, 0)
                            # drip deferred work into this batch's stream:
                            # b0 gets batch-1's V projection, b1 gets
                            # batch-0's out-projection.
                            if drip_delay > 0:
                                drip_delay -= 1
                            elif b == 0:
                                for _ in range(4):
                                    emit_vproj_mm()
                            else:
                                for _ in range(7):
                                    emit_outproj_mm()
                    # drain the last pending iteration
                    pqb, pkb, ps, j, n = pend
                    at = emit_exp(b, pqb, pkb, ps, j, n)
                    emit_ctx(b, pqb, pkb, pc, at, n)
                    nc.vector.tensor_copy(
                        ctxT[b][:, pqb * QB:(pqb + 1) * QB], pc[:]
                    )
                    fire_a2a(b, 1)
                    if b == 0:
                        # finish any V-projection work before b1's ctx needs it
                        while vq:
                            emit_vproj_mm()
                    if dbg:
                        nc.sync.dma_start(
                            dbg_ctxT[b * 128:(b + 1) * 128, :], ctxT[b][:]
                        )
                    # queue this batch's out-projection micro-ops; b=0's are
                    # dripped into b=1's attention, b=1's drain at the tail.
                    for h in range(2):
                        for fo in range(8):
                            for k in range(8):
                                oq.append((b, h, fo, k))

                # tail: whatever out-projection work is still queued
                while oq:
                    emit_outproj_mm()

    nc.compile()
    return nc


def kernel(**inputs):
    x = np.asarray(inputs["x"], np.float32)
    Wq = np.asarray(inputs["Wq"], np.float32)
    Wk = np.asarray(inputs["Wk"], np.float32)
    Wv = np.asarray(inputs["Wv"], np.float32)
    Wo = np.asarray(inputs["Wo"], np.float32)
    bo = np.asarray(inputs["bo"], np.float32)
    # bq/bk/bv are zeros by the problem's input spec; dif = x @ (Wq - Wk)
    # and v = x @ Wv absorb them exactly when zero.

    if "nc" not in _cached:
        _cached["nc"] = _build()
    nc = _cached["nc"]

    xT = np.ascontiguousarray(x.reshape(T, H).T).astype(np.float16)
    Wob = Wo.astype(ml_dtypes.bfloat16)
    maskbf = np.tril(np.ones((128, 128), np.float32), -1).astype(ml_dtypes.bfloat16)

    in_maps = []
    for c in range(NC):
        cols = slice(c * 128, (c + 1) * 128)
        in_maps.append({
            "xT": xT,
            "Wqc": np.ascontiguousarray(Wq[:, cols]),
            "Wkc": np.ascontiguousarray(Wk[:, cols]),
            "Wvc": np.ascontiguousarray(Wv[:, cols]).astype(np.float16),
            "Wob": Wob,
            "bo": bo,
            "maskbf": maskbf,
        })

    res = run_bass_kernel_spmd(nc, in_maps, core_ids=list(range(NC)))

    out = np.empty((B, S, H), np.float32)
    for c in range(NC):
        oT = res.results[c]["out"]                    # [H, 512]
        for b in range(B):
            for h in range(2):
                out[b, h * 1024 + c * OSLOT:h * 1024 + (c + 1) * OSLOT, :] = (
                    oT[:, (2 * b + h) * OSLOT:(2 * b + h + 1) * OSLOT].T
                )
    return out
